# revision 10
# baseline (speedup 1.0000x reference)
"""Chopfield attention v4 — host-fused query projection + fp8 DoubleRow cross terms.

Identities:
    Z = BETA*Re(conj(Q) @ K^T) = Re( conj(R) @ G @ Y^T ),
    G = BETA*conj(W_Q) @ W_K^T  (weight-only product, precomputed on host).

Precision scheme (validated numerically at rel_err ~5.5e-4):
  - hi/lo fp16 split of every score-chain operand; the hi*hi pass runs in
    fp16 (exact products, fp32 PSUM accumulate).
  - the two cross terms hi*lo + lo*hi are packed into ONE fp8e4m3 DoubleRow
    matmul per k-tile (2 weights/cell): slot0 = h8 * (l8*1024), slot1 =
    (l8*1024) * h8, accumulated in a separate PSUM bank and combined on DVE
    as z = hh + cross/1024.  Halves the PE cycles of the score chain.
  - G is scaled by BETA/2 so |Q''| < 240 fits fp8e4m3; the softmax exp uses
    scale=2 to compensate.
  - V path: single-pass fp16 Karatsuba (unchanged).
"""

import numpy as np
import ml_dtypes

import concourse.bacc as bacc
import concourse.mybir as mybir
import concourse.tile as tile
from concourse.bass_utils import run_bass_kernel_spmd

BETA = 0.03125
P = 128
FP16 = mybir.dt.float16
FP32 = mybir.dt.float32
F8 = mybir.dt.float8e4
E4 = ml_dtypes.float8_e4m3
X = mybir.AxisListType.X
DR = mybir.MatmulPerfMode.DoubleRow
S8 = 1024.0  # fp8 lo-term scale
SV = 2.0     # fp8 V/prob pair scale


class Cfg:
    def __init__(self, N=4096, M=4096, D=1024, NC=8):
        self.N, self.M, self.D, self.NC = N, M, D, NC
        self.NL = N // NC          # local query rows
        self.ML = M // NC          # local key rows (V path)
        self.DT = D // P           # contraction tiles
        self.QTS = self.NL // P    # local query partition-tiles
        self.MTS = self.ML // P    # local key partition-tiles
        self.DF = min(512, D)      # free-dim chunk for D-wide outputs
        self.DCH = D // self.DF
        self.MTG = M // P          # global key partition-tiles
        self.MF = 512              # score key-chunk width
        self.MCH = M // self.MF    # score key chunks
        self.SLOT = D * self.ML    # elements per gathered V tensor slot


def build(cfg: Cfg, reps: int = 1, no_collective: bool = False, stop_after: str | None = None):
    c = cfg
    nc = bacc.Bacc("TRN2", target_bir_lowering=False, debug=False, num_devices=c.NC)

    def din(name, shape, dt=FP16):
        return nc.dram_tensor(name, shape, dt, kind="ExternalInput")

    # V path: local Y^T shard + V weights
    ytl = {n: din(f"ytl_{n}", [P, c.DT * c.ML]) for n in ("re", "im", "s")}
    wv = {n: din(f"wv_{n}", [c.DCH, P, c.DT * c.DF]) for n in ("re", "im", "s")}
    # Q'' path: local R^T hi (fp16) + fp8 pair blocks; fused G weights
    rt = {comp: din(f"rt_{comp}_h", [P, c.DT * c.NL]) for comp in ("re", "im", "s")}
    rt8 = {comp: din(f"rt8_{comp}", [P, c.DT * 2 * c.NL], F8) for comp in ("re", "im", "s")}
    g = {comp: din(f"g_{comp}_h", [c.DT, P, c.DT * P]) for comp in ("re", "im", "d")}
    g8 = {comp: din(f"g8_{comp}", [c.DT, P, c.DT * 2 * P], F8) for comp in ("re", "im", "d")}
    # score moving operand: FULL Y^T hi (fp16) + fp8 pair blocks
    ytf = {comp: din(f"ytf_{comp}_h", [c.MCH, P, c.DT * c.MF]) for comp in ("re", "im")}
    ytf8 = {comp: din(f"ytf8_{comp}", [c.MCH, P, c.DT * 2 * c.MF], F8) for comp in ("re", "im")}

    ident = din("ident", [P, P])

    o_re = nc.dram_tensor("o_re", [c.NL, c.D], FP32, kind="ExternalOutput")
    o_im = nc.dram_tensor("o_im", [c.NL, c.D], FP32, kind="ExternalOutput")

    with tile.TileContext(nc) as tc:
        with (
            tc.tile_pool(name="pers", bufs=1) as pers,
            tc.tile_pool(name="ps", bufs=1, space="PSUM") as ps,
            tc.tile_pool(name="dram", bufs=1, space="DRAM") as dram,
        ):
            def emit(rep):
                # L pool slots [P, DT*NL] f16 (6 fresh), aliased through phases:
                #   rt_{re,im,s}_h : R^T hi loads -> p_sb[0..2] -> ptq[1..3]
                #   q_re_h/q_nim_h : Q'' hi (score stationary); q_re_h -> ptq[0]
                #   p3             : p_sb[3]
                # fp8 pair tiles: rt8_* (inputs), q8_* (device-built)
                L = tc.alloc_tile_pool(name=f"L{rep}", bufs=1)

                def Lt(tag):
                    return L.tile([P, c.DT * c.NL], FP16, tag=tag, name=f"{tag}_t{rep}")

                ident_sb = pers.tile([P, P], FP16, tag="ident")
                nc.sync.dma_start(ident_sb[:], ident.ap())
                cm = [L.tile([P, c.MCH], FP32, tag=f"cm{qt}", name=f"cm{qt}_{rep}") for qt in range(c.QTS)]
                ncm = [L.tile([P, c.MCH], FP32, tag=f"ncm{qt}", name=f"ncm{qt}_{rep}") for qt in range(c.QTS)]
                recip = [L.tile([P, 1], FP32, tag=f"rcp{qt}", name=f"rcp{qt}_{rep}") for qt in range(c.QTS)]

                agv_in = dram.tile([2 * c.SLOT * 2], F8)
                agv_out = dram.tile([c.NC * 2 * c.SLOT * 2], F8, addr_space="Shared")

                # ---------- V projection (single-pass fp16 Karatsuba) + AG(V) early
                kvp = tc.alloc_tile_pool(name=f"kvp{rep}", bufs=1)
                ytls = {}
                for n, t in ytl.items():
                    ytls[n] = kvp.tile([P, c.DT * c.ML], FP16, tag=f"ytl{n}", name=f"ytl{n}_{rep}")
                    nc.gpsimd.dma_start(ytls[n][:], t.ap())
                rts = {}
                rt8s = {}
                for comp in ("re", "im", "s"):
                    rts[comp] = Lt(f"rt_{comp}_h")
                    nc.gpsimd.dma_start(rts[comp][:], rt[comp].ap())
                    rt8s[comp] = L.tile([P, c.DT * 2 * c.NL], F8, tag=f"rt8_{comp}",
                                        name=f"rt8_{comp}_{rep}")
                    nc.gpsimd.dma_start(rt8s[comp][:], rt8[comp].ap())
                vp = tc.alloc_tile_pool(name=f"vp{rep}", bufs=1)
                wvidx = {"re": 0, "im": 1, "s": 2}
                for dch in range(c.DCH):
                    wvsl = vp.tile([P, 3 * c.DT * c.DF], FP16, tag="wvsl", bufs=2)
                    for wn, wi in wvidx.items():
                        nc.sync.dma_start(
                            wvsl[:, wi * c.DT * c.DF : (wi + 1) * c.DT * c.DF],
                            wv[wn].ap()[dch],
                        )
                    for mt in range(c.MTS):
                        m = {}
                        for prod, yc in enumerate(("re", "im", "s")):
                            pt = ps.tile([P, 512], FP32, tag="ps", bufs=6)
                            m[prod] = pt[:, : c.DF]
                            for ki in range(c.DT):
                                nc.tensor.matmul(
                                    m[prod],
                                    ytls[yc][:, ki * c.ML + mt * P : ki * c.ML + (mt + 1) * P],
                                    wvsl[:, wvidx[yc] * c.DT * c.DF + ki * c.DF : wvidx[yc] * c.DT * c.DF + (ki + 1) * c.DF],
                                    start=(ki == 0),
                                    stop=(ki == c.DT - 1),
                                )
                        vm2s = vp.tile([P, c.DF], FP32, tag="vm2s", bufs=2)
                        nc.vector.tensor_copy(vm2s[:], m[1])
                        for comp, si in (("re", 0), ("im", 1)):
                            vout = vp.tile([P, c.DF], FP16, tag="vout", bufs=4)
                            if comp == "re":
                                nc.vector.tensor_sub(vout[:], m[0], vm2s[:])
                            else:
                                vim1 = vp.tile([P, c.DF], FP32, tag="vim1", bufs=2)
                                nc.vector.tensor_sub(vim1[:], m[2], vm2s[:])
                                nc.vector.tensor_sub(vout[:], vim1[:], m[0])
                            v8 = vp.tile([P, 2 * c.DF], F8, tag="v8", bufs=4)
                            nc.scalar.activation(
                                v8[:, : c.DF], vout[:],
                                mybir.ActivationFunctionType.Copy, scale=1.0,
                            )
                            vl16 = vp.tile([P, c.DF], FP16, tag="vl16", bufs=2)
                            nc.vector.tensor_sub(vl16[:], vout[:], v8[:, : c.DF])
                            nc.scalar.activation(
                                v8[:, c.DF :], vl16[:],
                                mybir.ActivationFunctionType.Copy, scale=SV,
                            )
                            dst = agv_in[si * 2 * c.SLOT : (si + 1) * 2 * c.SLOT].rearrange(
                                "(m p dc two d) -> m p dc two d", m=c.MTS, p=P, dc=c.DCH, two=2
                            )[mt, :, dch, :, :]
                            nc.gpsimd.dma_start(dst, v8[:].rearrange("p (two d) -> p two d", two=2))
                if not no_collective:
                    nc.gpsimd.collective_compute(
                        "AllGather",
                        mybir.AluOpType.bypass,
                        replica_groups=[list(range(c.NC))],
                        ins=[agv_in.opt()],
                        outs=[agv_out.opt()],
                    )
                vp.release()
                kvp.release()
                if stop_after == "vproj":
                    L.release()
                    return

                # ---------- Q'' projection: Q''^T = G^T @ conj(R)^T
                # products m1 = Rre@Gre, m2 = Rim@Gim, m3 = (Rre+Rim)@(Gre-Gim)
                # re = m1+m2 ; nim = m3-m1+m2
                # each product: fp16 hh pass + fp8 DR cross pass (separate PSUM)
                q_sb = {"re": Lt("q_re_h"), "nim": Lt("q_nim_h")}
                q8_sb = {comp: L.tile([P, c.DT * 2 * c.NL], F8, tag=f"q8_{comp}",
                                      name=f"q8_{comp}_{rep}") for comp in ("re", "nim")}
                rsel = {"re": "re", "im": "im", "d": "s"}
                qp = tc.alloc_tile_pool(name=f"qp{rep}", bufs=1)
                gidx = {"re": 0, "im": 1, "d": 2}
                for ko in range(c.DT):
                    wsl = qp.tile([P, 3 * c.DT * P], FP16, tag="gsl", bufs=2)
                    w8sl = qp.tile([P, 3 * c.DT * 2 * P], F8, tag="g8sl", bufs=2)
                    for wc, wi in gidx.items():
                        nc.sync.dma_start(
                            wsl[:, wi * c.DT * P : (wi + 1) * c.DT * P], g[wc].ap()[ko]
                        )
                        nc.sync.dma_start(
                            w8sl[:, wi * c.DT * 2 * P : (wi + 1) * c.DT * 2 * P],
                            g8[wc].ap()[ko],
                        )

                    mc = {}
                    for prod, comp in enumerate(("re", "im", "d")):
                        wi = gidx[comp]
                        psA = ps.tile([P, 512], FP32, tag="ps", bufs=6)
                        psB = ps.tile([P, 512], FP32, tag="ps", bufs=6)
                        mA, mB = psA[:, : c.NL], psB[:, : c.NL]
                        for ki in range(c.DT):
                            nc.tensor.matmul(
                                mA,
                                wsl[:, wi * c.DT * P + ki * P : wi * c.DT * P + (ki + 1) * P],
                                rts[rsel[comp]][:, ki * c.NL : (ki + 1) * c.NL],
                                start=(ki == 0),
                                stop=(ki == c.DT - 1),
                            )
                        w8r = w8sl[:, wi * c.DT * 2 * P : (wi + 1) * c.DT * 2 * P].rearrange(
                            "p (ki two q) -> p ki two q", ki=c.DT, two=2
                        )
                        r8r = rt8s[rsel[comp]].rearrange(
                            "p (ki two n) -> p ki two n", ki=c.DT, two=2
                        )
                        for ki in range(c.DT):
                            nc.tensor.matmul(
                                mB,
                                w8r[:, ki],
                                r8r[:, ki],
                                start=(ki == 0),
                                stop=(ki == c.DT - 1),
                                perf_mode=DR,
                            )
                        csb = qp.tile([P, c.NL], FP32, tag=f"cs{prod}", bufs=2, name=f"cs{prod}_{rep}")
                        nc.scalar.activation(
                            csb[:], mB, mybir.ActivationFunctionType.Copy, scale=1.0 / S8
                        )
                        mc[prod] = qp.tile([P, c.NL], FP32, tag=f"mc{prod}", bufs=2, name=f"mc{prod}_{rep}")
                        nc.vector.tensor_add(mc[prod][:], mA, csb[:])
                    dre = qp.tile([P, c.NL], FP32, tag="qdre", bufs=2)
                    nc.vector.tensor_add(dre[:], mc[0][:], mc[1][:])
                    dnim = qp.tile([P, c.NL], FP32, tag="qdnim", bufs=2)
                    nc.vector.tensor_add(dnim[:], mc[2][:], mc[1][:])
                    nc.vector.tensor_sub(dnim[:], dnim[:], mc[0][:])
                    for comp, d in (("re", dre), ("nim", dnim)):
                        hi = q_sb[comp][:, ko * c.NL : (ko + 1) * c.NL]
                        nc.vector.tensor_copy(hi, d[:])
                        lo = qp.tile([P, c.NL], FP16, tag="qlo", bufs=2)
                        nc.vector.tensor_sub(lo[:], d[:], hi)
                        q8h = q8_sb[comp][:, ko * 2 * c.NL : ko * 2 * c.NL + c.NL]
                        q8l = q8_sb[comp][:, ko * 2 * c.NL + c.NL : (ko + 1) * 2 * c.NL]
                        nc.scalar.activation(
                            q8h, d[:], mybir.ActivationFunctionType.Copy, scale=1.0
                        )
                        nc.scalar.activation(
                            q8l, lo[:], mybir.ActivationFunctionType.Copy, scale=S8
                        )
                qp.release()
                if stop_after == "qproj":
                    L.release()
                    return

                # ---------- scores + streaming softmax over key chunks
                # z = Z/2 (G carries BETA/2); exp uses scale=2 to compensate
                p_sb = [Lt(tag) for tag in ("rt_re_h", "rt_im_h", "rt_s_h", "p3")]
                ptq = {}
                scp = tc.alloc_tile_pool(name=f"scp{rep}", bufs=1)
                yh_idx = {"re": 0, "im": 1}
                for mch in range(c.MCH):
                    ytile = scp.tile([P, 2 * c.DT * c.MF], FP16, tag="ytile", bufs=2)
                    yt8 = scp.tile([P, 2 * c.DT * 2 * c.MF], F8, tag="yt8", bufs=2)
                    for yc, yi in yh_idx.items():
                        nc.sync.dma_start(
                            ytile[:, yi * c.DT * c.MF : (yi + 1) * c.DT * c.MF],
                            ytf[yc].ap()[mch],
                        )
                        nc.sync.dma_start(
                            yt8[:, yi * c.DT * 2 * c.MF : (yi + 1) * c.DT * 2 * c.MF],
                            ytf8[yc].ap()[mch],
                        )

                    for qt in range(c.QTS):
                        psA = ps.tile([P, 512], FP32, tag="ps", bufs=6)
                        psB = ps.tile([P, 512], FP32, tag="ps", bufs=6)
                        zA, zB = psA[:, : c.MF], psB[:, : c.MF]
                        i = 0
                        for comp, yc in (("re", "re"), ("nim", "im")):
                            yi = yh_idx[yc]
                            for ki in range(c.DT):
                                nc.tensor.matmul(
                                    zA,
                                    q_sb[comp][:, ki * c.NL + qt * P : ki * c.NL + (qt + 1) * P],
                                    ytile[:, yi * c.DT * c.MF + ki * c.MF : yi * c.DT * c.MF + (ki + 1) * c.MF],
                                    start=(i == 0),
                                    stop=(i == 2 * c.DT - 1),
                                )
                                i += 1
                        i = 0
                        for comp, yc in (("re", "re"), ("nim", "im")):
                            yi = yh_idx[yc]
                            q8r = q8_sb[comp].rearrange(
                                "p (ki two n) -> p ki two n", ki=c.DT, two=2
                            )
                            y8r = yt8[:, yi * c.DT * 2 * c.MF : (yi + 1) * c.DT * 2 * c.MF].rearrange(
                                "p (ki two mf) -> p ki two mf", ki=c.DT, two=2
                            )
                            for ki in range(c.DT):
                                nc.tensor.matmul(
                                    zB,
                                    q8r[:, ki, :, qt * P : (qt + 1) * P],
                                    y8r[:, ki],
                                    start=(i == 0),
                                    stop=(i == 2 * c.DT - 1),
                                    perf_mode=DR,
                                )
                                i += 1
                        csc = scp.tile([P, c.MF], FP32, tag="csc", bufs=3)
                        nc.scalar.activation(
                            csc[:], zB, mybir.ActivationFunctionType.Copy, scale=1.0 / S8
                        )
                        zc = scp.tile([P, c.MF], FP32, tag="zc", bufs=3)
                        nc.vector.tensor_add(zc[:], zA, csc[:])
                        nc.vector.reduce_max(cm[qt][:, mch : mch + 1], zc[:], axis=X)
                        nc.vector.tensor_scalar_mul(
                            ncm[qt][:, mch : mch + 1], cm[qt][:, mch : mch + 1], -2.0
                        )
                        nc.scalar.activation(
                            p_sb[qt][:, mch * c.MF : (mch + 1) * c.MF],
                            zc[:],
                            mybir.ActivationFunctionType.Exp,
                            bias=ncm[qt][:, mch : mch + 1],
                            scale=2.0,
                        )
                        if mch == c.MCH - 1:
                            # finalize this query tile immediately: rescale
                            # chunks to the global max, rowsum, and DMA-xbar
                            # transpose P -> P^T — overlaps later qt's matmuls
                            ngm = L.tile([P, 1], FP32, tag=f"ngm{qt}", name=f"ngm{qt}_{rep}")
                            nc.vector.tensor_reduce(
                                ngm[:], ncm[qt][:], op=mybir.AluOpType.min, axis=X
                            )
                            fac = L.tile([P, c.MCH], FP32, tag=f"fac{qt}", name=f"fac{qt}_{rep}")
                            nc.scalar.activation(
                                fac[:],
                                ncm[qt][:],
                                mybir.ActivationFunctionType.Exp,
                                bias=ngm[:, 0:1],
                                scale=-1.0,
                            )
                            for mc2 in range(c.MCH):
                                nc.vector.tensor_scalar_mul(
                                    p_sb[qt][:, mc2 * c.MF : (mc2 + 1) * c.MF],
                                    p_sb[qt][:, mc2 * c.MF : (mc2 + 1) * c.MF],
                                    fac[:, mc2 : mc2 + 1],
                                )
                            ssum = L.tile([P, 1], FP32, tag=f"ssum{qt}", name=f"ssum{qt}_{rep}")
                            nc.vector.reduce_sum(ssum[:], p_sb[qt][:], axis=X)
                            nc.vector.reciprocal(recip[qt][:], ssum[:])
                            nc.vector.tensor_scalar_mul(recip[qt][:], recip[qt][:], 2.0)
                            # slot chaining: qt0 -> dead q slot; qt>=1 -> slot
                            # of the already-transposed p_sb[qt-1]
                            ptq[qt] = Lt(("q_re_h", "rt_re_h", "rt_im_h", "rt_s_h")[qt])
                            nc.sync.dma_start_transpose(
                                ptq[qt].rearrange("p (mtg q) -> p mtg q", mtg=c.MTG),
                                p_sb[qt][:],
                            )
                scp.release()
                if stop_after == "scores":
                    L.release()
                    return

                # ---------- A @ V via fp8 DoubleRow pairs (+ 2/sum scaling)
                avp = tc.alloc_tile_pool(name=f"avp{rep}", bufs=1)
                pt8 = {}
                for qt in range(c.QTS):
                    pt8[qt] = avp.tile([P, 2 * c.MTG * P], F8, tag=f"pt8_{qt}",
                                       name=f"pt8_{qt}_{rep}")
                    nc.vector.tensor_copy(pt8[qt][:, : c.MTG * P], ptq[qt][:])
                    nc.scalar.activation(
                        pt8[qt][:, c.MTG * P :], ptq[qt][:],
                        mybir.ActivationFunctionType.Copy, scale=1.0 / SV,
                    )
                for comp, odram in (("re", o_re), ("im", o_im)):
                    si = 0 if comp == "re" else 1
                    for dch in range(c.DCH):
                        vh = avp.tile([P, c.MTG * 2 * c.DF], F8, tag="vh", bufs=2)
                        for r in range(c.NC):
                            src8 = agv_out[
                                r * 4 * c.SLOT + si * 2 * c.SLOT : r * 4 * c.SLOT + (si + 1) * 2 * c.SLOT
                            ].rearrange("(m p dc two d) -> dc p m two d", m=c.MTS, p=P, dc=c.DCH, two=2)[dch]
                            nc.scalar.dma_start(
                                vh[
                                    :, r * c.MTS * 2 * c.DF : (r + 1) * c.MTS * 2 * c.DF
                                ].rearrange("p (m two d) -> p m two d", m=c.MTS, two=2),
                                src8,
                            )
                        vh4 = vh.rearrange("p (mtg two d) -> p mtg two d", mtg=c.MTG, two=2)
                        for qt in range(c.QTS):
                            pt8r = pt8[qt].rearrange("p (two x) -> p two x", two=2)
                            op_ = ps.tile([P, 512], FP32, tag="ps", bufs=6)
                            oacc = op_[:, : c.DF]
                            for mtg in range(c.MTG):
                                nc.tensor.matmul(
                                    oacc,
                                    pt8r[:, :, mtg * P : (mtg + 1) * P],
                                    vh4[:, mtg],
                                    start=(mtg == 0),
                                    stop=(mtg == c.MTG - 1),
                                    perf_mode=DR,
                                )
                            osb = avp.tile([P, c.DF], FP32, tag="osb", bufs=4)
                            nc.vector.tensor_scalar_mul(osb[:], oacc, recip[qt][:, 0:1])
                            nc.sync.dma_start(
                                odram.ap()[
                                    qt * P : (qt + 1) * P, dch * c.DF : (dch + 1) * c.DF
                                ],
                                osb[:],
                            )
                avp.release()
                L.release()

            for rep in range(reps):
                emit(rep)

    nc.compile()
    return nc


def _split16(x):
    h = x.astype(np.float16)
    l = (x - h.astype(np.float32)).astype(np.float16)
    return h, l


def _to8(x):
    a = np.asarray(x, dtype=np.float32)
    assert np.abs(a).max() < 240.0, f"fp8e4m3 overflow: {np.abs(a).max()}"
    return a.astype(E4)


def _pair8(h, l, order, S=1024.0):
    """Stack hi/lo fp8 blocks along a new slot axis (axis=-2 of tiles).

    order='hl': slot0 = h8, slot1 = l8*S (stationary side)
    order='lh': slot0 = l8*S, slot1 = h8 (moving side)
    """
    h8 = _to8(h)
    l8 = _to8(np.asarray(l, dtype=np.float32) * S)
    return (h8, l8) if order == "hl" else (l8, h8)


def prep_inputs(cfg, R_re, R_im, Y_re, Y_im, W_Q_re, W_Q_im, W_K_re, W_K_im, W_V_re, W_V_im):
    """Host-side sharding + fp16 hi/lo + fp8 pair prep. Returns in_maps."""
    c = cfg
    f32 = np.float32
    f64 = np.float64
    DT, DCH, DF, MCH, MF, NL = c.DT, c.DCH, c.DF, c.MCH, c.MF, c.NL

    def _wsw(w16, ocols):
        # [d_in, d_out] -> [d_out_block, p, d_in_tile * ocols], contiguous
        ob = w16.shape[1] // ocols
        return np.ascontiguousarray(
            w16.reshape(DT, P, ob, ocols).transpose(2, 1, 0, 3).reshape(ob, P, DT * ocols)
        )

    def _wsw8(h, l, ocols):
        # like _wsw but fp8 pairs: [d_out_block, p, d_in_tile * 2 * ocols]
        s0, s1 = _pair8(h, l, "hl")
        ob = h.shape[1] // ocols
        st = np.stack([s0, s1], axis=0)  # [2, d_in, d_out]
        return np.ascontiguousarray(
            st.reshape(2, DT, P, ob, ocols)
            .transpose(3, 2, 1, 0, 4)
            .reshape(ob, P, DT * 2 * ocols)
        )

    # fused query-side weight: G = (BETA/2) * conj(W_Q) @ W_K^T  (host, fp64)
    A = np.asarray(W_Q_re, dtype=f64)
    B = np.asarray(W_Q_im, dtype=f64)
    C = np.asarray(W_K_re, dtype=f64)
    D = np.asarray(W_K_im, dtype=f64)
    half_beta = BETA / 2.0
    g_re = (half_beta * (A @ C.T + B @ D.T)).astype(f32)
    g_im = (half_beta * (A @ D.T - B @ C.T)).astype(f32)

    wv_re = np.ascontiguousarray(W_V_re, dtype=f32) * 0.5
    wv_im = np.ascontiguousarray(W_V_im, dtype=f32) * 0.5

    shared = {}
    for comp, arr in (("re", g_re), ("im", g_im), ("d", g_re - g_im)):
        h, l = _split16(arr)
        shared[f"g_{comp}_h"] = _wsw(h, P)
        shared[f"g8_{comp}"] = _wsw8(h.astype(f32), l.astype(f32), P)
    shared["wv_re"] = _wsw(wv_re.astype(np.float16), DF)
    shared["wv_im"] = _wsw(wv_im.astype(np.float16), DF)
    shared["wv_s"] = _wsw((wv_re + wv_im).astype(np.float16), DF)
    shared["ident"] = np.eye(P, dtype=np.float16)

    # full Y^T for scores: fp16 hi [MCH, P, DT*MF] + fp8 pairs [MCH, P, DT*2*MF]
    for comp, arr in (("re", Y_re), ("im", Y_im)):
        t = np.ascontiguousarray(np.asarray(arr, dtype=f32).T)  # [D, M]
        h, l = _split16(t)
        shared[f"ytf_{comp}_h"] = np.ascontiguousarray(
            h.reshape(DT, P, MCH, MF).transpose(2, 1, 0, 3).reshape(MCH, P, DT * MF)
        )
        s0, s1 = _pair8(h.astype(f32), l.astype(f32), "lh")
        st = np.stack([s0, s1], axis=0)  # [2, D, M]
        shared[f"ytf8_{comp}"] = np.ascontiguousarray(
            st.reshape(2, DT, P, MCH, MF)
            .transpose(3, 2, 1, 0, 4)
            .reshape(MCH, P, DT * 2 * MF)
        )

    in_maps = []
    for r in range(c.NC):
        m = dict(shared)
        rsl = slice(r * c.NL, (r + 1) * c.NL)
        ysl = slice(r * c.ML, (r + 1) * c.ML)
        rre_t = np.ascontiguousarray(np.asarray(R_re[rsl], dtype=f32).T)
        rim_t = np.ascontiguousarray(np.asarray(R_im[rsl], dtype=f32).T)
        yre_t = np.ascontiguousarray(np.asarray(Y_re[ysl], dtype=f32).T)
        yim_t = np.ascontiguousarray(np.asarray(Y_im[ysl], dtype=f32).T)
        for base, arr in (("re", rre_t), ("im", rim_t), ("s", rre_t + rim_t)):
            h, l = _split16(arr)
            m[f"rt_{base}_h"] = np.ascontiguousarray(
                h.reshape(DT, P, NL).transpose(1, 0, 2).reshape(P, DT * NL)
            )
            s0, s1 = _pair8(h.astype(f32), l.astype(f32), "lh")
            st = np.stack([s0, s1], axis=0)  # [2, D, NL]
            m[f"rt8_{base}"] = np.ascontiguousarray(
                st.reshape(2, DT, P, NL).transpose(2, 1, 0, 3).reshape(P, DT * 2 * NL)
            )
        for n, arr in (("re", yre_t), ("im", yim_t), ("s", yre_t + yim_t)):
            mw = arr.shape[1]
            m[f"ytl_{n}"] = np.ascontiguousarray(
                arr.astype(np.float16).reshape(DT, P, mw).transpose(1, 0, 2).reshape(P, DT * mw)
            )
        in_maps.append(m)
    return in_maps


_NC_CACHE = {}


def kernel(**inputs) -> np.ndarray:
    cfg = Cfg()
    if "full" not in _NC_CACHE:
        _NC_CACHE["full"] = build(cfg, 1)
    nc = _NC_CACHE["full"]
    in_maps = prep_inputs(cfg, **inputs)
    res = run_bass_kernel_spmd(nc, in_maps, list(range(cfg.NC)))
    o_re = np.concatenate([res.results[r]["o_re"] for r in range(cfg.NC)], axis=0)
    o_im = np.concatenate([res.results[r]["o_im"] for r in range(cfg.NC)], axis=0)
    return (o_re + 1j * o_im).astype(np.complex64)


# revision 15
# speedup vs baseline: 1.5801x; 1.5801x over previous
"""Chopfield attention v4 — host-fused query projection + fp8 DoubleRow cross terms.

Identities:
    Z = BETA*Re(conj(Q) @ K^T) = Re( conj(R) @ G @ Y^T ),
    G = BETA*conj(W_Q) @ W_K^T  (weight-only product, precomputed on host).

Precision scheme (validated numerically at rel_err ~5.5e-4):
  - hi/lo fp16 split of every score-chain operand; the hi*hi pass runs in
    fp16 (exact products, fp32 PSUM accumulate).
  - the two cross terms hi*lo + lo*hi are packed into ONE fp8e4m3 DoubleRow
    matmul per k-tile (2 weights/cell): slot0 = h8 * (l8*1024), slot1 =
    (l8*1024) * h8, accumulated in a separate PSUM bank and combined on DVE
    as z = hh + cross/1024.  Halves the PE cycles of the score chain.
  - G is scaled by BETA/2 so |Q''| < 240 fits fp8e4m3; the softmax exp uses
    scale=2 to compensate.
  - V path: single-pass fp16 Karatsuba (unchanged).
"""

import numpy as np
import ml_dtypes

import concourse.bacc as bacc
import concourse.mybir as mybir
import concourse.tile as tile
from concourse.bass_utils import run_bass_kernel_spmd

BETA = 0.03125
P = 128
FP16 = mybir.dt.float16
FP32 = mybir.dt.float32
F8 = mybir.dt.float8e4
E4 = ml_dtypes.float8_e4m3
X = mybir.AxisListType.X
DR = mybir.MatmulPerfMode.DoubleRow
S8 = 1024.0  # fp8 lo-term scale
SV = 2.0     # fp8 V/prob pair scale


class Cfg:
    def __init__(self, N=4096, M=4096, D=1024, NC=8):
        self.N, self.M, self.D, self.NC = N, M, D, NC
        self.NL = N // NC          # local query rows
        self.ML = M // NC          # local key rows (V path)
        self.DT = D // P           # contraction tiles
        self.QTS = self.NL // P    # local query partition-tiles
        self.MTS = self.ML // P    # local key partition-tiles
        self.DF = min(512, D)      # free-dim chunk for D-wide outputs
        self.DCH = D // self.DF
        self.MTG = M // P          # global key partition-tiles
        self.MF = 512              # score key-chunk width
        self.MCH = M // self.MF    # score key chunks
        self.SLOT = D * self.ML    # elements per gathered V tensor slot


def build(cfg: Cfg, reps: int = 1, no_collective: bool = False, stop_after: str | None = None):
    c = cfg
    nc = bacc.Bacc("TRN2", target_bir_lowering=False, debug=False, num_devices=c.NC)

    def din(name, shape, dt=FP16):
        return nc.dram_tensor(name, shape, dt, kind="ExternalInput")

    # V path: local Y^T shard + V weights
    ytl = {n: din(f"ytl_{n}", [P, c.DT * c.ML]) for n in ("re", "im", "s")}
    wv = {n: din(f"wv_{n}", [c.DCH, P, c.DT * c.DF]) for n in ("re", "im", "s")}
    # Q'' path: local R^T hi (fp16) + fp8 pair blocks; fused G weights
    rt = {comp: din(f"rt_{comp}_h", [P, c.DT * c.NL]) for comp in ("re", "im", "s")}
    rt8 = {comp: din(f"rt8_{comp}", [P, c.DT * 2 * c.NL], F8) for comp in ("re", "im", "s")}
    g = {comp: din(f"g_{comp}_h", [c.DT, P, c.DT * P]) for comp in ("re", "im", "d")}
    g8 = {comp: din(f"g8_{comp}", [c.DT, P, c.DT * 2 * P], F8) for comp in ("re", "im", "d")}
    # score moving operand: FULL Y^T hi (fp16) + fp8 pair blocks
    ytf = {comp: din(f"ytf_{comp}_h", [c.MCH, P, c.DT * c.MF]) for comp in ("re", "im")}
    ytf8 = {comp: din(f"ytf8_{comp}", [c.MCH, P, c.DT * 2 * c.MF], F8) for comp in ("re", "im")}

    ident = din("ident", [P, P])

    o_re = nc.dram_tensor("o_re", [c.NL, c.D], FP32, kind="ExternalOutput")
    o_im = nc.dram_tensor("o_im", [c.NL, c.D], FP32, kind="ExternalOutput")

    with tile.TileContext(nc) as tc:
        with (
            tc.tile_pool(name="pers", bufs=1) as pers,
            tc.tile_pool(name="ps", bufs=1, space="PSUM") as ps,
            tc.tile_pool(name="dram", bufs=1, space="DRAM") as dram,
        ):
            def emit(rep):
                # L pool slots [P, DT*NL] f16 (6 fresh), aliased through phases:
                #   rt_{re,im,s}_h : R^T hi loads -> p_sb[0..2] -> ptq[1..3]
                #   q_re_h/q_nim_h : Q'' hi (score stationary); q_re_h -> ptq[0]
                #   p3             : p_sb[3]
                # fp8 pair tiles: rt8_* (inputs), q8_* (device-built)
                L = tc.alloc_tile_pool(name=f"L{rep}", bufs=1)

                def Lt(tag):
                    return L.tile([P, c.DT * c.NL], FP16, tag=tag, name=f"{tag}_t{rep}")

                ident_sb = pers.tile([P, P], FP16, tag="ident")
                nc.sync.dma_start(ident_sb[:], ident.ap())
                cm = [L.tile([P, c.MCH], FP32, tag=f"cm{qt}", name=f"cm{qt}_{rep}") for qt in range(c.QTS)]
                ncm = [L.tile([P, c.MCH], FP32, tag=f"ncm{qt}", name=f"ncm{qt}_{rep}") for qt in range(c.QTS)]
                recip = [L.tile([P, 1], FP32, tag=f"rcp{qt}", name=f"rcp{qt}_{rep}") for qt in range(c.QTS)]
                esum = [L.tile([P, c.MCH], FP32, tag=f"esum{qt}", name=f"esum{qt}_{rep}") for qt in range(c.QTS)]

                g0 = L.tile([P, 3 * c.DT * P], FP16, tag="g0", name=f"g0_{rep}")
                g80 = L.tile([P, 3 * c.DT * 2 * P], F8, tag="g80", name=f"g80_{rep}")
                for wc, wi in (("re", 0), ("im", 1), ("d", 2)):
                    nc.sync.dma_start(
                        g0[:, wi * c.DT * P : (wi + 1) * c.DT * P], g[wc].ap()[0]
                    )
                    nc.sync.dma_start(
                        g80[:, wi * c.DT * 2 * P : (wi + 1) * c.DT * 2 * P],
                        g8[wc].ap()[0],
                    )

                agv_in = dram.tile([2 * c.SLOT * 2], F8)
                agv_out = dram.tile([c.NC * 2 * c.SLOT * 2], F8, addr_space="Shared")

                # ---------- V projection (single-pass fp16 Karatsuba) + AG(V) early
                kvp = tc.alloc_tile_pool(name=f"kvp{rep}", bufs=1)
                ytls = {}
                for n, t in ytl.items():
                    ytls[n] = kvp.tile([P, c.DT * c.ML], FP16, tag=f"ytl{n}", name=f"ytl{n}_{rep}")
                    nc.gpsimd.dma_start(ytls[n][:], t.ap())
                rts = {}
                rt8s = {}
                for comp in ("re", "im", "s"):
                    rts[comp] = Lt(f"rt_{comp}_h")
                    nc.gpsimd.dma_start(rts[comp][:], rt[comp].ap())
                    rt8s[comp] = L.tile([P, c.DT * 2 * c.NL], F8, tag=f"rt8_{comp}",
                                        name=f"rt8_{comp}_{rep}")
                    nc.gpsimd.dma_start(rt8s[comp][:], rt8[comp].ap())
                vp = tc.alloc_tile_pool(name=f"vp{rep}", bufs=1)
                wvidx = {"re": 0, "im": 1, "s": 2}
                for dch in range(c.DCH):
                    wvsl = vp.tile([P, 3 * c.DT * c.DF], FP16, tag="wvsl", bufs=2)
                    for wn, wi in wvidx.items():
                        nc.sync.dma_start(
                            wvsl[:, wi * c.DT * c.DF : (wi + 1) * c.DT * c.DF],
                            wv[wn].ap()[dch],
                        )
                    for mt in range(c.MTS):
                        m = {}
                        for prod, yc in enumerate(("re", "im", "s")):
                            pt = ps.tile([P, 512], FP32, tag="ps", bufs=8)
                            m[prod] = pt[:, : c.DF]
                            for ki in range(c.DT):
                                nc.tensor.matmul(
                                    m[prod],
                                    ytls[yc][:, ki * c.ML + mt * P : ki * c.ML + (mt + 1) * P],
                                    wvsl[:, wvidx[yc] * c.DT * c.DF + ki * c.DF : wvidx[yc] * c.DT * c.DF + (ki + 1) * c.DF],
                                    start=(ki == 0),
                                    stop=(ki == c.DT - 1),
                                )
                        vm2s = vp.tile([P, c.DF], FP32, tag="vm2s", bufs=2)
                        nc.vector.tensor_copy(vm2s[:], m[1])
                        for comp, si in (("re", 0), ("im", 1)):
                            vout = vp.tile([P, c.DF], FP16, tag="vout", bufs=4)
                            if comp == "re":
                                nc.vector.tensor_sub(vout[:], m[0], vm2s[:])
                            else:
                                vim1 = vp.tile([P, c.DF], FP32, tag="vim1", bufs=2)
                                nc.vector.tensor_sub(vim1[:], m[2], vm2s[:])
                                nc.vector.tensor_sub(vout[:], vim1[:], m[0])
                            v8 = vp.tile([P, 2 * c.DF], F8, tag="v8", bufs=4)
                            nc.scalar.activation(
                                v8[:, : c.DF], vout[:],
                                mybir.ActivationFunctionType.Copy, scale=1.0,
                            )
                            vl16 = vp.tile([P, c.DF], FP16, tag="vl16", bufs=2)
                            nc.vector.tensor_sub(vl16[:], vout[:], v8[:, : c.DF])
                            nc.scalar.activation(
                                v8[:, c.DF :], vl16[:],
                                mybir.ActivationFunctionType.Copy, scale=SV,
                            )
                            dst = agv_in[si * 2 * c.SLOT : (si + 1) * 2 * c.SLOT].rearrange(
                                "(m p dc two d) -> m p dc two d", m=c.MTS, p=P, dc=c.DCH, two=2
                            )[mt, :, dch, :, :]
                            nc.gpsimd.dma_start(dst, v8[:].rearrange("p (two d) -> p two d", two=2))
                if not no_collective:
                    nc.gpsimd.collective_compute(
                        "AllGather",
                        mybir.AluOpType.bypass,
                        replica_groups=[list(range(c.NC))],
                        ins=[agv_in.opt()],
                        outs=[agv_out.opt()],
                    )
                vp.release()
                kvp.release()
                if stop_after == "vproj":
                    L.release()
                    return

                # ---------- Q'' projection: Q''^T = G^T @ conj(R)^T
                # products m1 = Rre@Gre, m2 = Rim@Gim, m3 = (Rre+Rim)@(Gre-Gim)
                # re = m1+m2 ; nim = m3-m1+m2
                # each product: fp16 hh pass + fp8 DR cross pass (separate PSUM)
                q_sb = {"re": Lt("q_re_h"), "nim": Lt("q_nim_h")}
                q8_sb = {comp: L.tile([P, c.DT * 2 * c.NL], F8, tag=f"q8_{comp}",
                                      name=f"q8_{comp}_{rep}") for comp in ("re", "nim")}
                rsel = {"re": "re", "im": "im", "d": "s"}
                qpe = tc.alloc_tile_pool(name=f"qpe{rep}", bufs=1)
                qp = tc.alloc_tile_pool(name=f"qp{rep}", bufs=1)
                gidx = {"re": 0, "im": 1, "d": 2}
                for ko in range(c.DT):
                    if ko == 0:
                        wsl, w8sl = g0, g80
                    else:
                        wsl = qp.tile([P, 3 * c.DT * P], FP16, tag="gsl", bufs=2)
                        w8sl = qp.tile([P, 3 * c.DT * 2 * P], F8, tag="g8sl", bufs=2)
                        for wc, wi in gidx.items():
                            nc.sync.dma_start(
                                wsl[:, wi * c.DT * P : (wi + 1) * c.DT * P], g[wc].ap()[ko]
                            )
                            nc.sync.dma_start(
                                w8sl[:, wi * c.DT * 2 * P : (wi + 1) * c.DT * 2 * P],
                                g8[wc].ap()[ko],
                            )

                    mc = {}
                    for prod, comp in enumerate(("re", "im", "d")):
                        wi = gidx[comp]
                        psA = ps.tile([P, 512], FP32, tag="ps", bufs=8)
                        psB = ps.tile([P, 512], FP32, tag="ps", bufs=8)
                        mA, mB = psA[:, : c.NL], psB[:, : c.NL]
                        for ki in range(c.DT):
                            nc.tensor.matmul(
                                mA,
                                wsl[:, wi * c.DT * P + ki * P : wi * c.DT * P + (ki + 1) * P],
                                rts[rsel[comp]][:, ki * c.NL : (ki + 1) * c.NL],
                                start=(ki == 0),
                                stop=(ki == c.DT - 1),
                            )
                        w8r = w8sl[:, wi * c.DT * 2 * P : (wi + 1) * c.DT * 2 * P].rearrange(
                            "p (ki two q) -> p ki two q", ki=c.DT, two=2
                        )
                        r8r = rt8s[rsel[comp]].rearrange(
                            "p (ki two n) -> p ki two n", ki=c.DT, two=2
                        )
                        for ki in range(c.DT):
                            nc.tensor.matmul(
                                mB,
                                w8r[:, ki],
                                r8r[:, ki],
                                start=(ki == 0),
                                stop=(ki == c.DT - 1),
                                perf_mode=DR,
                            )
                        csb = qpe.tile([P, c.NL], FP32, tag=f"cs{prod}", bufs=2, name=f"cs{prod}_{rep}")
                        nc.scalar.activation(
                            csb[:], mB, mybir.ActivationFunctionType.Copy, scale=1.0 / S8
                        )
                        mc[prod] = qpe.tile([P, c.NL], FP32, tag=f"mc{prod}", bufs=2, name=f"mc{prod}_{rep}")
                        nc.vector.tensor_add(mc[prod][:], mA, csb[:])
                    dre = qpe.tile([P, c.NL], FP32, tag="qdre", bufs=2)
                    nc.vector.tensor_add(dre[:], mc[0][:], mc[1][:])
                    dnim = qpe.tile([P, c.NL], FP32, tag="qdnim", bufs=2)
                    nc.vector.tensor_add(dnim[:], mc[2][:], mc[1][:])
                    nc.vector.tensor_sub(dnim[:], dnim[:], mc[0][:])
                    for comp, d in (("re", dre), ("nim", dnim)):
                        hi = q_sb[comp][:, ko * c.NL : (ko + 1) * c.NL]
                        nc.vector.tensor_copy(hi, d[:])
                        lo = qpe.tile([P, c.NL], FP16, tag="qlo", bufs=2)
                        nc.vector.tensor_sub(lo[:], d[:], hi)
                        q8h = q8_sb[comp][:, ko * 2 * c.NL : ko * 2 * c.NL + c.NL]
                        q8l = q8_sb[comp][:, ko * 2 * c.NL + c.NL : (ko + 1) * 2 * c.NL]
                        nc.scalar.activation(
                            q8h, d[:], mybir.ActivationFunctionType.Copy, scale=1.0
                        )
                        nc.scalar.activation(
                            q8l, lo[:], mybir.ActivationFunctionType.Copy, scale=S8
                        )
                qp.release()
                qpe.release()
                if stop_after == "qproj":
                    L.release()
                    return

                # ---------- scores + streaming softmax over key chunks
                # z = Z/2 (G carries BETA/2); exp uses scale=2 to compensate
                p_sb = [Lt(tag) for tag in ("rt_re_h", "rt_im_h", "rt_s_h", "p3")]
                ptq = {}
                pt8 = {}
                ptp = tc.alloc_tile_pool(name=f"ptp{rep}", bufs=1)
                scp = tc.alloc_tile_pool(name=f"scp{rep}", bufs=1)
                yh_idx = {"re": 0, "im": 1}
                for mch in range(c.MCH):
                    ytile = scp.tile([P, 2 * c.DT * c.MF], FP16, tag="ytile", bufs=2)
                    yt8 = scp.tile([P, 2 * c.DT * 2 * c.MF], F8, tag="yt8", bufs=2)
                    for yc, yi in yh_idx.items():
                        nc.sync.dma_start(
                            ytile[:, yi * c.DT * c.MF : (yi + 1) * c.DT * c.MF],
                            ytf[yc].ap()[mch],
                        )
                        nc.sync.dma_start(
                            yt8[:, yi * c.DT * 2 * c.MF : (yi + 1) * c.DT * 2 * c.MF],
                            ytf8[yc].ap()[mch],
                        )

                    for qt in range(c.QTS):
                        psA = ps.tile([P, 512], FP32, tag="ps", bufs=8)
                        psB = ps.tile([P, 512], FP32, tag="ps", bufs=8)
                        zA, zB = psA[:, : c.MF], psB[:, : c.MF]
                        i = 0
                        for comp, yc in (("re", "re"), ("nim", "im")):
                            yi = yh_idx[yc]
                            for ki in range(c.DT):
                                nc.tensor.matmul(
                                    zA,
                                    q_sb[comp][:, ki * c.NL + qt * P : ki * c.NL + (qt + 1) * P],
                                    ytile[:, yi * c.DT * c.MF + ki * c.MF : yi * c.DT * c.MF + (ki + 1) * c.MF],
                                    start=(i == 0),
                                    stop=(i == 2 * c.DT - 1),
                                )
                                i += 1
                        i = 0
                        for comp, yc in (("re", "re"), ("nim", "im")):
                            yi = yh_idx[yc]
                            q8r = q8_sb[comp].rearrange(
                                "p (ki two n) -> p ki two n", ki=c.DT, two=2
                            )
                            y8r = yt8[:, yi * c.DT * 2 * c.MF : (yi + 1) * c.DT * 2 * c.MF].rearrange(
                                "p (ki two mf) -> p ki two mf", ki=c.DT, two=2
                            )
                            for ki in range(c.DT):
                                nc.tensor.matmul(
                                    zB,
                                    q8r[:, ki, :, qt * P : (qt + 1) * P],
                                    y8r[:, ki],
                                    start=(i == 0),
                                    stop=(i == 2 * c.DT - 1),
                                    perf_mode=DR,
                                )
                                i += 1
                        csc = scp.tile([P, c.MF], FP32, tag="csc", bufs=2)
                        nc.scalar.activation(
                            csc[:], zB, mybir.ActivationFunctionType.Copy, scale=1.0 / S8
                        )
                        zc = scp.tile([P, c.MF], FP32, tag="zc", bufs=2)
                        nc.vector.tensor_add(zc[:], zA, csc[:])
                        nc.vector.reduce_max(cm[qt][:, mch : mch + 1], zc[:], axis=X)
                        nc.vector.tensor_scalar_mul(
                            ncm[qt][:, mch : mch + 1], cm[qt][:, mch : mch + 1], -2.0
                        )
                        nc.scalar.activation(
                            p_sb[qt][:, mch * c.MF : (mch + 1) * c.MF],
                            zc[:],
                            mybir.ActivationFunctionType.Exp,
                            bias=ncm[qt][:, mch : mch + 1],
                            scale=2.0,
                            accum_out=esum[qt][:, mch : mch + 1],
                        )
                        if mch == c.MCH - 1:
                            # finalize this query tile immediately: rescale
                            # chunks to the global max, rowsum, and DMA-xbar
                            # transpose P -> P^T — overlaps later qt's matmuls
                            ngm = L.tile([P, 1], FP32, tag=f"ngm{qt}", name=f"ngm{qt}_{rep}")
                            nc.vector.tensor_reduce(
                                ngm[:], ncm[qt][:], op=mybir.AluOpType.min, axis=X
                            )
                            fac = L.tile([P, c.MCH], FP32, tag=f"fac{qt}", name=f"fac{qt}_{rep}")
                            nc.scalar.activation(
                                fac[:],
                                ncm[qt][:],
                                mybir.ActivationFunctionType.Exp,
                                bias=ngm[:, 0:1],
                                scale=-1.0,
                            )
                            for mc2 in range(c.MCH):
                                nc.vector.tensor_scalar_mul(
                                    p_sb[qt][:, mc2 * c.MF : (mc2 + 1) * c.MF],
                                    p_sb[qt][:, mc2 * c.MF : (mc2 + 1) * c.MF],
                                    fac[:, mc2 : mc2 + 1],
                                )
                            ssum = L.tile([P, 1], FP32, tag=f"ssum{qt}", name=f"ssum{qt}_{rep}")
                            fs = L.tile([P, c.MCH], FP32, tag=f"fs{qt}", name=f"fs{qt}_{rep}")
                            nc.vector.scalar_tensor_tensor(
                                fs[:], esum[qt][:], 1.0, fac[:],
                                op0=mybir.AluOpType.mult, op1=mybir.AluOpType.mult,
                                accum_out=ssum[:],
                            )
                            nc.vector.reciprocal(recip[qt][:], ssum[:])
                            nc.vector.tensor_scalar_mul(recip[qt][:], recip[qt][:], 2.0)
                            # slot chaining: qt0 -> dead q slot; qt>=1 -> slot
                            # of the already-transposed p_sb[qt-1]
                            ptq[qt] = Lt(("q_re_h", "rt_re_h", "rt_im_h", "rt_s_h")[qt])
                            nc.sync.dma_start_transpose(
                                ptq[qt].rearrange("p (mtg q) -> p mtg q", mtg=c.MTG),
                                p_sb[qt][:],
                            )
                            pt8[qt] = ptp.tile([P, 2 * c.MTG * P], F8, tag=f"pt8_{qt}",
                                               name=f"pt8_{qt}_{rep}")
                            nc.gpsimd.tensor_copy(pt8[qt][:, : c.MTG * P], ptq[qt][:])
                            nc.scalar.activation(
                                pt8[qt][:, c.MTG * P :], ptq[qt][:],
                                mybir.ActivationFunctionType.Copy, scale=1.0 / SV,
                            )
                scp.release()
                if stop_after == "scores":
                    L.release()
                    return

                # ---------- A @ V via fp8 DoubleRow pairs (+ 2/sum scaling)
                avp = tc.alloc_tile_pool(name=f"avp{rep}", bufs=1)
                for comp, odram in (("re", o_re), ("im", o_im)):
                    si = 0 if comp == "re" else 1
                    for dch in range(c.DCH):
                        vh = avp.tile([P, c.MTG * 2 * c.DF], F8, tag="vh", bufs=2)
                        for r in range(c.NC):
                            src8 = agv_out[
                                r * 4 * c.SLOT + si * 2 * c.SLOT : r * 4 * c.SLOT + (si + 1) * 2 * c.SLOT
                            ].rearrange("(m p dc two d) -> dc p m two d", m=c.MTS, p=P, dc=c.DCH, two=2)[dch]
                            q_eng = nc.sync if r % 2 == 0 else nc.scalar
                            q_eng.dma_start(
                                vh[
                                    :, r * c.MTS * 2 * c.DF : (r + 1) * c.MTS * 2 * c.DF
                                ].rearrange("p (m two d) -> p m two d", m=c.MTS, two=2),
                                src8,
                            )
                        vh4 = vh.rearrange("p (mtg two d) -> p mtg two d", mtg=c.MTG, two=2)
                        for qt in range(c.QTS):
                            pt8r = pt8[qt].rearrange("p (two x) -> p two x", two=2)
                            op_ = ps.tile([P, 512], FP32, tag="ps", bufs=8)
                            oacc = op_[:, : c.DF]
                            for mtg in range(c.MTG):
                                nc.tensor.matmul(
                                    oacc,
                                    pt8r[:, :, mtg * P : (mtg + 1) * P],
                                    vh4[:, mtg],
                                    start=(mtg == 0),
                                    stop=(mtg == c.MTG - 1),
                                    perf_mode=DR,
                                )
                            osb = avp.tile([P, c.DF], FP32, tag="osb", bufs=4)
                            nc.vector.tensor_scalar_mul(osb[:], oacc, recip[qt][:, 0:1])
                            nc.sync.dma_start(
                                odram.ap()[
                                    qt * P : (qt + 1) * P, dch * c.DF : (dch + 1) * c.DF
                                ],
                                osb[:],
                            )
                avp.release()
                ptp.release()
                L.release()

            for rep in range(reps):
                emit(rep)

    nc.compile()
    return nc


def _split16(x):
    h = x.astype(np.float16)
    l = (x - h.astype(np.float32)).astype(np.float16)
    return h, l


def _to8(x):
    a = np.asarray(x, dtype=np.float32)
    assert np.abs(a).max() < 240.0, f"fp8e4m3 overflow: {np.abs(a).max()}"
    return a.astype(E4)


def _pair8(h, l, order, S=1024.0):
    """Stack hi/lo fp8 blocks along a new slot axis (axis=-2 of tiles).

    order='hl': slot0 = h8, slot1 = l8*S (stationary side)
    order='lh': slot0 = l8*S, slot1 = h8 (moving side)
    """
    h8 = _to8(h)
    l8 = _to8(np.asarray(l, dtype=np.float32) * S)
    return (h8, l8) if order == "hl" else (l8, h8)


def prep_inputs(cfg, R_re, R_im, Y_re, Y_im, W_Q_re, W_Q_im, W_K_re, W_K_im, W_V_re, W_V_im):
    """Host-side sharding + fp16 hi/lo + fp8 pair prep. Returns in_maps."""
    c = cfg
    f32 = np.float32
    f64 = np.float64
    DT, DCH, DF, MCH, MF, NL = c.DT, c.DCH, c.DF, c.MCH, c.MF, c.NL

    def _wsw(w16, ocols):
        # [d_in, d_out] -> [d_out_block, p, d_in_tile * ocols], contiguous
        ob = w16.shape[1] // ocols
        return np.ascontiguousarray(
            w16.reshape(DT, P, ob, ocols).transpose(2, 1, 0, 3).reshape(ob, P, DT * ocols)
        )

    def _wsw8(h, l, ocols):
        # like _wsw but fp8 pairs: [d_out_block, p, d_in_tile * 2 * ocols]
        s0, s1 = _pair8(h, l, "hl")
        ob = h.shape[1] // ocols
        st = np.stack([s0, s1], axis=0)  # [2, d_in, d_out]
        return np.ascontiguousarray(
            st.reshape(2, DT, P, ob, ocols)
            .transpose(3, 2, 1, 0, 4)
            .reshape(ob, P, DT * 2 * ocols)
        )

    # fused query-side weight: G = (BETA/2) * conj(W_Q) @ W_K^T  (host, fp64)
    A = np.asarray(W_Q_re, dtype=f64)
    B = np.asarray(W_Q_im, dtype=f64)
    C = np.asarray(W_K_re, dtype=f64)
    D = np.asarray(W_K_im, dtype=f64)
    half_beta = BETA / 2.0
    g_re = (half_beta * (A @ C.T + B @ D.T)).astype(f32)
    g_im = (half_beta * (A @ D.T - B @ C.T)).astype(f32)

    wv_re = np.ascontiguousarray(W_V_re, dtype=f32) * 0.5
    wv_im = np.ascontiguousarray(W_V_im, dtype=f32) * 0.5

    shared = {}
    for comp, arr in (("re", g_re), ("im", g_im), ("d", g_re - g_im)):
        h, l = _split16(arr)
        shared[f"g_{comp}_h"] = _wsw(h, P)
        shared[f"g8_{comp}"] = _wsw8(h.astype(f32), l.astype(f32), P)
    shared["wv_re"] = _wsw(wv_re.astype(np.float16), DF)
    shared["wv_im"] = _wsw(wv_im.astype(np.float16), DF)
    shared["wv_s"] = _wsw((wv_re + wv_im).astype(np.float16), DF)
    shared["ident"] = np.eye(P, dtype=np.float16)

    # full Y^T for scores: fp16 hi [MCH, P, DT*MF] + fp8 pairs [MCH, P, DT*2*MF]
    for comp, arr in (("re", Y_re), ("im", Y_im)):
        t = np.ascontiguousarray(np.asarray(arr, dtype=f32).T)  # [D, M]
        h, l = _split16(t)
        shared[f"ytf_{comp}_h"] = np.ascontiguousarray(
            h.reshape(DT, P, MCH, MF).transpose(2, 1, 0, 3).reshape(MCH, P, DT * MF)
        )
        s0, s1 = _pair8(h.astype(f32), l.astype(f32), "lh")
        st = np.stack([s0, s1], axis=0)  # [2, D, M]
        shared[f"ytf8_{comp}"] = np.ascontiguousarray(
            st.reshape(2, DT, P, MCH, MF)
            .transpose(3, 2, 1, 0, 4)
            .reshape(MCH, P, DT * 2 * MF)
        )

    in_maps = []
    for r in range(c.NC):
        m = dict(shared)
        rsl = slice(r * c.NL, (r + 1) * c.NL)
        ysl = slice(r * c.ML, (r + 1) * c.ML)
        rre_t = np.ascontiguousarray(np.asarray(R_re[rsl], dtype=f32).T)
        rim_t = np.ascontiguousarray(np.asarray(R_im[rsl], dtype=f32).T)
        yre_t = np.ascontiguousarray(np.asarray(Y_re[ysl], dtype=f32).T)
        yim_t = np.ascontiguousarray(np.asarray(Y_im[ysl], dtype=f32).T)
        for base, arr in (("re", rre_t), ("im", rim_t), ("s", rre_t + rim_t)):
            h, l = _split16(arr)
            m[f"rt_{base}_h"] = np.ascontiguousarray(
                h.reshape(DT, P, NL).transpose(1, 0, 2).reshape(P, DT * NL)
            )
            s0, s1 = _pair8(h.astype(f32), l.astype(f32), "lh")
            st = np.stack([s0, s1], axis=0)  # [2, D, NL]
            m[f"rt8_{base}"] = np.ascontiguousarray(
                st.reshape(2, DT, P, NL).transpose(2, 1, 0, 3).reshape(P, DT * 2 * NL)
            )
        for n, arr in (("re", yre_t), ("im", yim_t), ("s", yre_t + yim_t)):
            mw = arr.shape[1]
            m[f"ytl_{n}"] = np.ascontiguousarray(
                arr.astype(np.float16).reshape(DT, P, mw).transpose(1, 0, 2).reshape(P, DT * mw)
            )
        in_maps.append(m)
    return in_maps


_NC_CACHE = {}


def kernel(**inputs) -> np.ndarray:
    cfg = Cfg()
    if "full" not in _NC_CACHE:
        _NC_CACHE["full"] = build(cfg, 1)
    nc = _NC_CACHE["full"]
    in_maps = prep_inputs(cfg, **inputs)
    res = run_bass_kernel_spmd(nc, in_maps, list(range(cfg.NC)))
    o_re = np.concatenate([res.results[r]["o_re"] for r in range(cfg.NC)], axis=0)
    o_im = np.concatenate([res.results[r]["o_im"] for r in range(cfg.NC)], axis=0)
    return (o_re + 1j * o_im).astype(np.complex64)
